# revision 43
# baseline (speedup 1.0000x reference)
"""Trainium2 Bass kernel for nn_Neural_time_50337016709696 (embedding_lookup + RFF).

Computation (reference):
    Uvec[b] = concat_m U[m, b_i_n[b, m]]            # [B, 96] gather
    x[b]    = concat(Uvec[b], t[b])                 # [B, 97]
    proj    = x @ W_freq                            # [B, 256]
    y       = (sin(proj)@w1 + cos(proj)@w2)/16 + b  # [B, 1]

Device strategy (data-parallel over batch, 8 cores, replicated tables):
  * Table stored f16 (halves gather bytes; ~0.2% rel err). Indirect DMA
    gathers pull U rows into the per-batch-row layout X[128p, g*96+...].
    HW consumes exactly one index per partition per instruction (probed:
    extra idx columns are ignored and consecutive rows are read instead),
    so it is one gather per idx column; these 384 instructions serialize
    on the Pool engine's software DGE and dominate the kernel.
  * PE transposes X tiles (f16, 1 cycle/row) into feature-major Xt (batch
    on free dim) via PSUM; ACT/DVE copy them out (split across engines).
    Xt gets three extra rows: t[b], 1.0 (phase row), 1.0 (+1536 row).
  * q' = (x@W_freq + phi)/(2pi) + 1536 via one f16 matmul per 128-feature
    chunk (scaling, phase phi, and the +1536 ride the weight rows;
    w1*sin(p)+w2*cos(p) = A*sin(p+phi) with A/phi precomputed on host).
    q' in [1024,2048) has a fixed fp32 exponent, so frac(q') is the low
    13 mantissa bits: one DVE tensor_scalar (AND 0x1FFF, OR 0x3F800000)
    yields g = 1 + frac/1024 -- a single-pass range reduction.
  * ACT Sin((pi-d) - (2pi-2d)*1024*(g-1)) = sin(proj+phi) to f16
    (d=1.5e-3 keeps the affine inside Sin's [-pi,pi] domain).
  * y = A_c.T @ s_c accumulated on PE (col-tiled over 4 PSUM positions);
    ACT adds b_out; strided-partition DMA writes y (host de-interleaves).
"""

import math

import numpy as np

import concourse.bass as bass
import concourse.mybir as mybir
import concourse.tile as tile
from concourse import bacc
from concourse.bass import IndirectOffsetOnAxis
from concourse.masks import make_identity

P = 128
R = 32
NMOD = 3
NVEC = 500000
NFF = 256
IN_DIM = NMOD * R + 1  # 97
IN2 = 98               # + phase row
IN3 = 99               # + sixteen row (ones-row paired with w=16.0)
N_CORES = 8
B = 131072
B_LOC = B // N_CORES  # 16384

F32 = mybir.dt.float32
F32R = mybir.dt.float32r
F16 = mybir.dt.float16
I32 = mybir.dt.int32

SIN = mybir.ActivationFunctionType.Sin
IDENT = mybir.ActivationFunctionType.Identity
TWO_PI = float(2.0 * math.pi)
GK = 1                          # idx columns per indirect-DMA gather;
                                # HW consumes exactly one index per
                                # partition per instruction (probed), so
                                # gk>1 is sim-only.
SIN_D = 1.5e-3                  # sin-domain compression margin (rad)


def emit_kernel(tc, u_ap, idx_ap, t1_ap, w_ap, amp_ap, b_ap, y_ap,
                b_loc=B_LOC, n_chunks=4, mm_dtype=F16, reps=1, gk=1,
                copy_eng="act"):
    """Emit the per-core program. All *_ap are DRAM APs:
      u_ap    [NMOD*nvec, R] f16   (gather table, mode-major)
      idx_ap  [128, 3*G] i32       idx[p, 3g+m] = b_i[g*128+p, m] + m*nvec
      t1_ap   [3, b_loc] f16       rows: b_t_n (natural order), 1.0, 1.0
      w_ap    [99, 256] f16        rows 0..96: W_freq/2pi; 97: phi/2pi; 98: 16
      amp_ap  [128, 64] f16        A=hypot(w1,w2); col 32c+j = A-chunk-c
      b_ap    [128, 1] f32         b_out replicated
      y_ap    [4, b_loc//4] f32    y_ap[q, 512*S+n] = y[2048*S + 512*q + n]
    """
    nc = tc.nc
    G = b_loc // P                  # groups of 128 batches
    GPB = 8                         # groups per compute block (1024 batches)
    assert G % GPB == 0
    NB = G // GPB
    assert NB % 2 == 0
    NS = NB // 2                    # superblocks of 2048 batches
    assert G % n_chunks == 0
    gpc = G // n_chunks             # groups per gather chunk
    assert gpc % GPB == 0
    bpc = gpc // GPB                # blocks per chunk

    from contextlib import ExitStack
    with ExitStack() as ctx:
        const_pool = ctx.enter_context(tc.tile_pool(name="const", bufs=1))
        x_pool = ctx.enter_context(tc.tile_pool(name="xdata", bufs=1))
        xt_pool = ctx.enter_context(tc.tile_pool(name="xt", bufs=2))
        z_pool = ctx.enter_context(tc.tile_pool(name="zfrac", bufs=2))
        s_pool = ctx.enter_context(tc.tile_pool(name="sins", bufs=2))
        y_pool = ctx.enter_context(tc.tile_pool(name="yout", bufs=2))
        pt_pool = ctx.enter_context(tc.tile_pool(name="ptr", bufs=2, space="PSUM"))
        pr_pool = ctx.enter_context(tc.tile_pool(name="prj", bufs=1, space="PSUM"))
        py_pool = ctx.enter_context(tc.tile_pool(name="pyy", bufs=2, space="PSUM"))

        # constants / inputs to SBUF
        idx_sb = const_pool.tile([P, 3 * G], I32, name="idx_sb")
        w_sb = const_pool.tile([IN3, NFF], mm_dtype, name="w_sb")
        amp_sb = const_pool.tile([P, 64], F16, name="amp_sb")
        b_sb = const_pool.tile([P, 1], F32, name="b_sb")
        pi_sb = const_pool.tile([P, 1], F32, name="pi_sb")
        ident = const_pool.tile([P, P], F16, name="ident")
        # arg = (pi-d) - (2pi-2d)*f spans [-pi+d, pi-d]: the d-compression
        # keeps the fp32-rounded affine strictly inside sin's domain at the
        # cost of a <=d phase warp (negligible at d=1.5e-3).
        nc.gpsimd.memset(pi_sb[:, :],
                         math.pi - SIN_D + (TWO_PI - 2 * SIN_D) * 1024.0)

        nc.sync.dma_start(out=idx_sb[:, :], in_=idx_ap)
        nc.sync.dma_start(out=w_sb[:, :], in_=w_ap)
        nc.sync.dma_start(out=amp_sb[:, :], in_=amp_ap)
        nc.sync.dma_start(out=b_sb[:, :], in_=b_ap)
        make_identity(nc, ident[:, :])

        # gather chunks: X_q [128, gpc*96]; X[p, 96 g + 32 m + e]
        #   = U[idx[p, 3g+m], e]
        # One indirect DMA consumes a [128, gk] block of the idx tile:
        # idx (p, k) fills out[p, 32k:32k+32] (flat orders line up), so
        # larger gk amortizes the per-instruction SWDGE overhead.
        def do_gathers(rep):
            x_tiles = []
            for q in range(n_chunks):
                xq = x_pool.tile([P, 96 * gpc], F16, tag=f"xq{q}",
                                 name=f"xq{q}_r{rep}")
                for j0 in range(0, 3 * gpc, gk):
                    j1 = min(j0 + gk, 3 * gpc)
                    jj = 3 * gpc * q + j0
                    nc.gpsimd.indirect_dma_start(
                        out=xq[:, 32 * j0:32 * j1],
                        out_offset=None,
                        in_=u_ap,
                        in_offset=IndirectOffsetOnAxis(
                            ap=idx_sb[:, jj:jj + (j1 - j0)], axis=0),
                    )
                x_tiles.append(xq)
            return x_tiles

        def do_block(x_tiles, rep, i, py, s_pos):
            """block i = 1024 batches; y accumulated into psum tile `py`
            at col-tile positions {2*s_pos, 2*s_pos+1}."""
            q = i // bpc
            xq = x_tiles[q]
            base_g = i * GPB - q * gpc

            xt = xt_pool.tile([IN3, 1024], mm_dtype, tag="xt",
                              name=f"xt{i}_r{rep}")
            # rows 96 (t), 97 (ones), 98 (ones for the +1536 weight row)
            nc.sync.dma_start(out=xt[96:99, :],
                              in_=t1_ap[0:3, 1024 * i:1024 * (i + 1)])
            for h in range(2):
                pt = pt_pool.tile([P, 512], F16, tag="pt",
                                  name=f"pt{i}_{h}_r{rep}")
                for gl in range(4):
                    g_in = base_g + h * 4 + gl
                    nc.tensor.transpose(
                        out=pt[0:96, 128 * gl:128 * (gl + 1)],
                        in_=xq[:, 96 * g_in:96 * (g_in + 1)],
                        identity=ident[:, :])
                # PSUM -> SBUF move; engine choice balances ACT vs DVE load
                use_dve = copy_eng == "dve" or (copy_eng == "split" and h == 0)
                if use_dve:
                    nc.vector.tensor_copy(
                        out=xt[0:96, 512 * h:512 * (h + 1)],
                        in_=pt[0:96, 0:512])
                else:
                    nc.scalar.activation(
                        out=xt[0:96, 512 * h:512 * (h + 1)],
                        in_=pt[0:96, 0:512], func=IDENT)

            sss = []
            for c in range(2):
                pc = pr_pool.tile([P, 1024], F32, tag=f"proj{c}",
                                  name=f"proj{c}_{i}_r{rep}")
                for h2 in range(2):
                    nc.tensor.matmul(
                        out=pc[:, 512 * h2:512 * (h2 + 1)],
                        lhsT=w_sb[0:IN3, 128 * c:128 * (c + 1)],
                        rhs=xt[0:IN3, 512 * h2:512 * (h2 + 1)],
                        start=True, stop=True)
                # q' = (proj+phi)/2pi + 1536 in [1024,2048): one DVE pass
                # extracts the 13 frac mantissa bits (AND) and repacks them
                # onto 1.0f (OR): g = 1 + frac(q')/1024 quantized to 2^-13.
                zc = z_pool.tile([P, 1024], I32, tag=f"z{c}",
                                 name=f"z{c}_{i}_r{rep}")
                nc.vector.tensor_scalar(
                    out=zc[:, :], in0=pc[:, :].bitcast(I32), scalar1=0x1FFF,
                    scalar2=0x3F800000, op0=mybir.AluOpType.bitwise_and,
                    op1=mybir.AluOpType.bitwise_or)
                # sin(proj+phi) = sin(2pi f) = Sin(pi - 2pi f)
                #              = Sin(g*(-2pi*1024) + pi + 2pi*1024) on ACT.
                sc = s_pool.tile([P, 1024], F16, tag=f"s{c}",
                                 name=f"s{c}_{i}_r{rep}")
                nc.scalar.activation(out=sc[:, :], in_=zc[:, :].bitcast(F32),
                                     func=SIN, bias=pi_sb[0:P, 0:1],
                                     scale=-(TWO_PI - 2 * SIN_D) * 1024.0)
                sss.append(sc)

            for h2 in range(2):
                pos = 2 * s_pos + h2
                for c in range(2):
                    nc.tensor.matmul(
                        out=py[32 * pos:32 * pos + 32, :],
                        lhsT=amp_sb[0:P, 32 * c:32 * (c + 1)],
                        rhs=sss[c][:, 512 * h2:512 * (h2 + 1)],
                        start=(c == 0), stop=(c == 1),
                        tile_position=(0, 32 * pos))

        for rep in range(reps):
            x_tiles = do_gathers(rep)
            for S in range(NS):
                py = py_pool.tile([P, 512], F32, tag="py",
                                  name=f"py{S}_r{rep}")
                do_block(x_tiles, rep, 2 * S, py, 0)
                do_block(x_tiles, rep, 2 * S + 1, py, 1)
                # full-width copy + b_out; DMA picks rows {0,32,64,96}
                ys = y_pool.tile([P, 512], F32, tag="ystage",
                                 name=f"ys{S}_r{rep}")
                nc.scalar.activation(out=ys[:, :], in_=py[:, :], func=IDENT,
                                     bias=b_sb[0:P, 0:1])
                nc.sync.dma_start(out=y_ap[0:4, 512 * S:512 * (S + 1)],
                                  in_=ys[0:128:32, 0:512])


def build_program(b_loc=B_LOC, nvec=NVEC, n_chunks=4, n_cores=N_CORES,
                  mm_dtype=F16, reps=1, gk=1, dma_scratch=None,
                  copy_eng="act"):
    """Build the full Bass module (one SPMD program for all cores)."""
    G = b_loc // P
    kw = {}
    if dma_scratch is None and gk > 8:
        # SWDGE descriptor ring must hold one instruction's 128*gk descs
        # (16B each), with headroom.
        dma_scratch = 16 * 128 * gk * 2
    if dma_scratch is not None:
        kw["dynamic_dma_scratch_size"] = dma_scratch
    nc = bacc.Bacc("TRN2", target_bir_lowering=False, debug=False,
                   num_devices=n_cores, name="rff_embed", **kw)
    u_d = nc.dram_tensor("u_tab", [NMOD * nvec, R], F16, kind="ExternalInput").ap()
    idx_d = nc.dram_tensor("idx", [P, 3 * G], I32, kind="ExternalInput").ap()
    t1_d = nc.dram_tensor("tvec", [3, b_loc], F16, kind="ExternalInput").ap()
    w_d = nc.dram_tensor("wfreq", [IN3, NFF], F16, kind="ExternalInput").ap()
    amp_d = nc.dram_tensor("amp", [P, 64], F16, kind="ExternalInput").ap()
    b_d = nc.dram_tensor("bout", [P, 1], F32, kind="ExternalInput").ap()
    y_d = nc.dram_tensor("y", [4, b_loc // 4], F32, kind="ExternalOutput").ap()

    with tile.TileContext(nc) as tc:
        emit_kernel(tc, u_d, idx_d, t1_d, w_d, amp_d, b_d, y_d,
                    b_loc=b_loc, n_chunks=n_chunks, mm_dtype=mm_dtype,
                    reps=reps, gk=gk, copy_eng=copy_eng)
    nc.compile()
    return nc


def prep_shared(U, W_freq, w_out, b_out, nvec=NVEC):
    """Host prep of the replicated tensors."""
    u_arr = np.ascontiguousarray(
        np.asarray(U, np.float16).reshape(NMOD * nvec, R))
    inv = 1.0 / np.sqrt(np.float64(NFF))
    w_out = np.asarray(w_out)
    w1 = w_out[:NFF, 0].astype(np.float64) * inv
    w2 = w_out[NFF:, 0].astype(np.float64) * inv
    amp = np.hypot(w1, w2)
    phi = np.arctan2(w2, w1)
    w_arr = np.empty((IN3, NFF), np.float64)
    w_arr[:IN_DIM] = np.asarray(W_freq, np.float64) / (2 * np.pi)
    w_arr[IN_DIM] = phi / (2 * np.pi)
    # +1536 pins q' = (proj+phi)/2pi + 1536 into [1024, 2048): fixed fp32
    # exponent, so frac(q') lives in the low 13 mantissa bits. A dedicated
    # ones-row keeps the offset exact in f16.
    w_arr[IN_DIM + 1] = 1536.0
    w_arr = np.ascontiguousarray(w_arr.astype(np.float16))
    amp_arr = np.empty((P, 64), np.float64)
    amp_arr[:, 0:32] = amp[:P, None]
    amp_arr[:, 32:64] = amp[P:, None]
    amp_arr = np.ascontiguousarray(amp_arr.astype(np.float16))
    b_arr = np.full((P, 1), np.asarray(b_out).reshape(()), np.float32)
    return u_arr, w_arr, amp_arr, b_arr


def prep_core(b_i, b_t, nvec=NVEC):
    """Host prep of one core's sharded index / t tensors."""
    b_loc = b_i.shape[0]
    G = b_loc // P
    offs = (np.arange(NMOD, dtype=np.int64) * nvec)
    idx = (np.asarray(b_i, np.int64).reshape(G, P, NMOD) + offs[None, None, :])
    idx = np.ascontiguousarray(
        idx.transpose(1, 0, 2).reshape(P, 3 * G).astype(np.int32))
    t1 = np.empty((3, b_loc), np.float16)
    t1[0] = np.asarray(b_t, np.float32).reshape(b_loc).astype(np.float16)
    t1[1] = 1.0
    t1[2] = 1.0
    return idx, t1


def unscramble_y(y_d, b_loc):
    """y_d [4, b_loc//4] -> y [b_loc] natural order."""
    ns = b_loc // 2048
    return np.ascontiguousarray(
        y_d.reshape(4, ns, 512).transpose(1, 0, 2).reshape(b_loc))


_PROGRAM_CACHE = {}


def kernel(b_i_n, b_t_n, U, W_freq, w_out, b_out):
    from concourse.bass_utils import run_bass_kernel_spmd

    key = "full"
    if key not in _PROGRAM_CACHE:
        _PROGRAM_CACHE[key] = build_program(gk=GK)
    nc = _PROGRAM_CACHE[key]

    u_arr, w_arr, amp_arr, b_arr = prep_shared(U, W_freq, w_out, b_out)
    in_maps = []
    for k in range(N_CORES):
        sl = slice(k * B_LOC, (k + 1) * B_LOC)
        idx, t1 = prep_core(np.asarray(b_i_n)[sl], np.asarray(b_t_n)[sl])
        in_maps.append({"u_tab": u_arr, "idx": idx, "tvec": t1,
                        "wfreq": w_arr, "amp": amp_arr, "bout": b_arr})

    res = run_bass_kernel_spmd(nc, in_maps, core_ids=list(range(N_CORES)))
    y = np.concatenate([unscramble_y(r["y"], B_LOC) for r in res.results])
    return y.reshape(B, 1).astype(np.float32)

